# revision 1
# baseline (speedup 1.0000x reference)
"""Tensor-parallel causal self-attention (GQA + RoPE) for 8 Trainium2 cores.

Sharding: heads across cores. Each core gets 4 query heads + 1 KV head
(wq cols c*256:(c+1)*256, wk/wv cols c*64:(c+1)*64, wo rows c*256:(c+1)*256).
Each core computes a full [S, H] partial output (f16); the host sums the 8
partials in f32.

Device-side layouts are all "transposed" (channels on partitions):
  qT [dim, seq], kT [dim, seq] -> scores^T tiles [j, i] (row-tiled head
  pairs run concurrently on the PE) -> exp on ACT -> 0/1 tri-mask multiply
  on the diagonal sub-tile (DVE, f16) -> PV matmul with lhsT = [v | ones]
  giving attn_out^T and softmax denominators in one accumulation.
Normalization: denominator row -> f16 -> ones-matmul broadcast to 128
partitions (PE) -> reciprocal_approx_fast (DVE) -> attn_out^T x recip
multiplies on GPSIMD (proxy library) -> aoT, which is exactly the lhsT
needed by the o_projection.
All matmuls run in fp16 (full PE rate).
"""

import json
import sys

import numpy as np

for _p in ("/opt/trn_rl_repo",):
    if _p not in sys.path:
        sys.path.insert(0, _p)

import concourse.bass as bass
import concourse.tile as tile
from concourse import mybir
from concourse.bass_utils import run_bass_kernel_spmd

B, S, H = 1, 2048, 2048
NH, NKV, HD = 32, 8, 64
ROPE_BASE = 10000.0
NCORES = 8
HQ = NH // NCORES            # 4 q heads per core
QW = HQ * HD                 # 256 q channels per core
NB = 512                     # xT streaming block width (seq positions)
IB = 512                     # attention i-block width
F32 = mybir.dt.float32
F16 = mybir.dt.float16
MMDT = F16                   # dtype for all matmul operands
MMNP = np.float16


def _split_multi_waits(bir_bytes: bytes) -> bytes:
    """This container's walrus accepts only one sync-wait per instruction;
    move extra waits onto preceding same-engine NoOps."""
    bir = json.loads(bir_bytes)
    n = [0]
    for fn in bir.get("functions", []):
        for bb in fn.get("blocks", []):
            insts = bb.get("instructions")
            if not insts:
                continue
            out = []
            for inst in insts:
                si = inst.get("sync_info")
                waits = (si or {}).get("on_wait") or []
                if len(waits) > 1:
                    for w in waits[:-1]:
                        n[0] += 1
                        out.append({
                            "debug": inst.get("debug", 0),
                            "engine": inst["engine"],
                            "ins": [], "outs": [],
                            "name": f"{inst['name']}-sw{n[0]}",
                            "opcode": "NoOp",
                            "sync_info": {"on_wait": [w], "on_update": []},
                        })
                    si["on_wait"] = waits[-1:]
                out.append(inst)
            bb["instructions"] = out
    return json.dumps(bir).encode()


def build_nc():
    nc = bass.Bass()
    KT = H // 128            # 16 contraction k-tiles for projections
    NBLK = S // NB           # 4 xT blocks
    IBLK = S // IB           # 4 attention i-blocks
    JTN = S // 128           # 16 key j-tiles
    CH = 2                   # proj matmuls per filler chunk

    xT = nc.dram_tensor("xT", [H, S], MMDT, kind="ExternalInput")
    wq = nc.dram_tensor("wq", [H, QW], MMDT, kind="ExternalInput")
    wkv = nc.dram_tensor("wkv", [H, 128], MMDT, kind="ExternalInput")
    wo = nc.dram_tensor("wo", [QW, H], MMDT, kind="ExternalInput")
    cosT = nc.dram_tensor("cosT", [128, S], MMDT, kind="ExternalInput")
    sinT = nc.dram_tensor("sinT", [128, S], MMDT, kind="ExternalInput")
    tri01_d = nc.dram_tensor("tri01", [128, 128], MMDT, kind="ExternalInput")
    ident64_d = nc.dram_tensor("ident64", [64, 64], MMDT, kind="ExternalInput")
    ones_d = nc.dram_tensor("ones", [128, 129], MMDT, kind="ExternalInput")
    out_d = nc.dram_tensor("out", [S, H], F16, kind="ExternalOutput")

    with tile.TileContext(nc) as tc:
        with (
            tc.tile_pool(name="const", bufs=1) as cpool,
            tc.tile_pool(name="xin", bufs=3) as xpool,
            tc.tile_pool(name="tmp", bufs=3) as tpool,
            tc.tile_pool(name="ex", bufs=4) as expool,
            tc.tile_pool(name="aon", bufs=2) as aopool,
            tc.tile_pool(name="ostage", bufs=4) as opool,
            tc.tile_pool(name="ps_pj", bufs=2, space="PSUM") as ps_pj,
            tc.tile_pool(name="ps_sc", bufs=2, space="PSUM") as ps_sc,
            tc.tile_pool(name="ps_pv", bufs=2, space="PSUM") as ps_pv,
        ):
            # ---- persistent SBUF ----
            wq_sb = cpool.tile([128, KT, QW], MMDT)
            wkv_sb = cpool.tile([128, KT, 128], MMDT)
            wo_sb = cpool.tile([128, 2, H], MMDT)
            cos_sb = cpool.tile([128, S], MMDT)
            sin_sb = cpool.tile([128, S], MMDT)
            tri_sb = cpool.tile([128, 128], MMDT)
            ident2 = cpool.tile([128, 64], MMDT)       # identity on rows 64:128
            onesr = cpool.tile([128, 129], MMDT)       # all-ones (row 64 = bc lhsT)
            qT_sb = cpool.tile([128, 2, S], MMDT)      # heads (0,1 | 2,3)
            kT_sb = cpool.tile([128, S], MMDT)         # kT duplicated on halves
            vT_sb = cpool.tile([128, S], MMDT)         # v^T on rows 64:128
            vnat_sb = cpool.tile([128, JTN, HD + 1], MMDT)
            aoT_sb = cpool.tile([128, 2, S], MMDT)     # attn_out^T (o_proj lhsT)

            def load_consts_pre():
                # Critical-path loads: ~256KB pieces, desc issue split across
                # the sync (xt, emitted by make_proj_chunks) and scalar queues
                # so descriptors issue in parallel and the first-needed pieces
                # lead each queue-sem's rotation.
                wqr = wq.rearrange("(ko p) c -> p ko c", p=128)
                for q in range(4):
                    nc.scalar.dma_start(wq_sb[:, 4 * q:4 * q + 4, :],
                                        wqr[:, 4 * q:4 * q + 4, :])
                wkr = wkv.rearrange("(ko p) c -> p ko c", p=128)
                for q in range(2):
                    nc.scalar.dma_start(wkv_sb[:, 8 * q:8 * q + 8, :],
                                        wkr[:, 8 * q:8 * q + 8, :])
                # cos/sin: load only the distinct partition rows from HBM
                # (cos rows repeat 4x, sin rows 2x), duplicate inside SBUF
                nc.sync.dma_start(cos_sb[0:32, :], cosT[0:32, :])
                nc.sync.dma_start(sin_sb[0:64, :], sinT[0:64, :])
                for b in (32, 64, 96):
                    nc.sync.dma_start(cos_sb[b:b + 32, :], cos_sb[0:32, :])
                nc.sync.dma_start(sin_sb[64:128, :], sin_sb[0:64, :])
                nc.scalar.dma_start(tri_sb[:], tri01_d[:])
                nc.scalar.dma_start(ident2[64:128, :], ident64_d[:])
                nc.scalar.dma_start(onesr[:], ones_d[:])
                nc.scalar.dma_start(vnat_sb[:, :, HD:HD + 1], ones_d[:, 0:JTN, None])

            def load_consts_post():
                wor = wo.rearrange("(t p) e -> p t e", p=128)
                for q in range(2):
                    nc.scalar.dma_start(wo_sb[:, q, :], wor[:, q, :])

            def rope(dst, src_sb, nb, rows=128):
                """dst = src*cos + rot(src)*sinS; rot via sbuf->sbuf DMA
                partition shuffle (sign baked into sinS)."""
                sl = bass.ts(nb, NB)
                rot = tpool.tile([128, NB], MMDT, tag="rot", name="rot")
                for b in ((0, 64) if rows == 128 else (0,)):
                    nc.sync.dma_start(rot[b:b + 32, :], src_sb[b + 32:b + 64, :])
                    nc.sync.dma_start(rot[b + 32:b + 64, :], src_sb[b:b + 32, :])
                m1 = tpool.tile([128, NB], MMDT, tag="m1", name="m1")
                nc.vector.tensor_tensor(m1[0:rows, :], src_sb[0:rows, :],
                                        cos_sb[0:rows, sl], mybir.AluOpType.mult)
                m2 = tpool.tile([128, NB], MMDT, tag="m2", name="m2")
                nc.vector.tensor_tensor(m2[0:rows, :], rot[0:rows, :],
                                        sin_sb[0:rows, sl], mybir.AluOpType.mult)
                nc.vector.tensor_tensor(dst, m1[0:rows, :], m2[0:rows, :],
                                        mybir.AluOpType.add)

            def make_proj_chunks(nb):
                """Prefetch xT block now; return callables that emit the
                projection matmuls/evictions/rope piecewise."""
                sl = bass.ts(nb, NB)
                xt = xpool.tile([128, KT, NB], MMDT, tag="xt", name=f"xt{nb}")
                xr = xT[:, sl].rearrange("(ko p) s -> p ko s", p=128)
                npc = 4 if nb <= 1 else 2
                for q in range(npc):
                    ks = KT // npc
                    nc.sync.dma_start(xt[:, ks * q:ks * (q + 1), :],
                                      xr[:, ks * q:ks * (q + 1), :])
                chunks = []
                for mt in range(3):  # 0,1 = q m-tiles; 2 = kv
                    pj = ps_pj.tile([128, NB], F32, tag="pj", name=f"pj_{nb}_{mt}")
                    w_sb = wkv_sb if mt == 2 else wq_sb

                    def mk_mm(k0, mt=mt, pj=pj, w_sb=w_sb):
                        def emit():
                            for k in range(k0, min(k0 + CH, KT)):
                                wsl = w_sb[:, k, :] if mt == 2 else \
                                    w_sb[:, k, bass.ts(mt, 128)]
                                nc.tensor.matmul(pj[:], wsl, xt[:, k, :],
                                                 start=(k == 0), stop=(k == KT - 1))
                        return emit
                    for k0 in range(0, KT, CH):
                        chunks.append(mk_mm(k0))

                    if mt < 2:
                        def ev(mt=mt, pj=pj):
                            qtmp = tpool.tile([128, NB], MMDT, tag="qtmp", name="qtmp")
                            nc.vector.tensor_copy(qtmp[:], pj[:])
                            rope(qT_sb[:, mt, sl], qtmp, nb)
                        chunks.append(ev)
                    else:
                        def evkv(pj=pj):
                            ktmp = tpool.tile([128, NB], MMDT, tag="ktmp", name="ktmp")
                            nc.vector.tensor_copy(ktmp[0:64, :], pj[0:64, :])
                            nc.vector.tensor_copy(vT_sb[64:128, sl], pj[64:128, :])
                            rope(kT_sb[0:64, sl], ktmp, nb, rows=64)
                            nc.sync.dma_start(kT_sb[64:128, sl], kT_sb[0:64, sl])
                        chunks.append(evkv)

                def tpc():
                    for jj in range(NB // 128):
                        jt = (nb * NB) // 128 + jj
                        tp_t = ps_sc.tile([128, 2, IB], MMDT, tag="sc", name="tp")
                        tp = tp_t[:, 0, :HD]
                        nc.tensor.transpose(tp[:], vT_sb[64:128, bass.ts(jt, 128)],
                                            ident2[64:128, :])
                        nc.vector.tensor_copy(vnat_sb[:, jt, 0:HD], tp[:])
                chunks.append(tpc)
                return chunks

            def make_oproj_chunks(it, og_on_act=False):
                chunks = []
                for sti in range(it * (IB // 128), (it + 1) * (IB // 128)):
                    og = opool.tile([128, 4, 512], F16, tag="og", name="og",
                                    bufs=4)
                    for eb in range(H // 512):
                        def opc(sti=sti, eb=eb, og=og):
                            ssl = bass.ts(sti, 128)
                            op = ps_pj.tile([128, 512], F32, tag="pj", name="op")
                            nc.tensor.matmul(op[:], aoT_sb[:, 0, ssl],
                                             wo_sb[:, 0, bass.ts(eb, 512)],
                                             start=True, stop=False)
                            nc.tensor.matmul(op[:], aoT_sb[:, 1, ssl],
                                             wo_sb[:, 1, bass.ts(eb, 512)],
                                             start=False, stop=True)
                            if og_on_act:
                                nc.scalar.copy(og[:, eb, :], op[:])
                            else:
                                nc.vector.tensor_copy(og[:, eb, :], op[:])
                            if eb == 3:  # one batched row-stripe DMA
                                nc.sync.dma_start(
                                    out_d[ssl, :].rearrange(
                                        "p (e c) -> p e c", e=4), og[:])
                        chunks.append(opc)
                return chunks

            def attention(it, chunks):
                i_lo = it * IB
                isl = bass.ts(it, IB)
                njt = (it + 1) * (IB // 128)
                dd = aopool.tile([128, 4 * IB], F16, tag="dd", name="dd",
                                 bufs=2)  # denominators live on row 64
                aos = []
                for mt in range(2):  # head pair on partitions 0:64 / 64:128
                    pv0 = ps_pv.tile([HD + 1, IB], F32, tag="pv", name="pv0")
                    pv1 = ps_pv.tile([HD + 1, IB], F32, tag="pv", name="pv1")
                    for jt in range(njt):
                        i0 = max(0, jt * 128 - i_lo)
                        st = ps_sc.tile([128, 2, IB], F32, tag="sc", name="st")
                        nc.tensor.matmul(
                            st[:, 0, i0:IB], kT_sb[0:64, bass.ts(jt, 128)],
                            qT_sb[0:64, mt, i_lo + i0:i_lo + IB],
                            start=True, stop=True)
                        nc.tensor.matmul(
                            st[:, 1, i0:IB], kT_sb[64:128, bass.ts(jt, 128)],
                            qT_sb[64:128, mt, i_lo + i0:i_lo + IB],
                            start=True, stop=True)
                        ex = expool.tile([128, 2, IB], MMDT, tag="ex", name="ex")
                        nc.scalar.activation(
                            ex[:, :, i0:IB], st[:, :, i0:IB],
                            mybir.ActivationFunctionType.Exp, scale=1.0 / 8.0)
                        if chunks:
                            chunks.pop(0)()  # PE filler while ACT runs exp
                        if jt * 128 >= i_lo:  # zero the upper-tri of diag tile
                            nc.vector.tensor_tensor(
                                ex[:, :, i0:i0 + 128], ex[:, :, i0:i0 + 128],
                                tri_sb[:, None, :].to_broadcast((128, 2, 128)),
                                mybir.AluOpType.mult)
                        nc.tensor.matmul(
                            pv0[:, i0:IB], vnat_sb[:, jt, :], ex[:, 0, i0:IB],
                            start=(jt == 0), stop=(jt == njt - 1))
                        nc.tensor.matmul(
                            pv1[:, i0:IB], vnat_sb[:, jt, :], ex[:, 1, i0:IB],
                            start=(jt == 0), stop=(jt == njt - 1))
                    # stage attn_out^T + denominators; pv slots free fast.
                    # Partition-aligned DVE copies; the odd-head shift to
                    # rows 64:128 rides a SBUF-SBUF DMA.
                    nc.vector.tensor_copy(
                        dd[64:65, 2 * mt * IB:(2 * mt + 1) * IB],
                        pv0[HD:HD + 1, :])
                    nc.vector.tensor_copy(
                        dd[64:65, (2 * mt + 1) * IB:(2 * mt + 2) * IB],
                        pv1[HD:HD + 1, :])
                    ao = aopool.tile([128, IB], F16, tag="ao", name="ao")
                    nc.vector.tensor_copy(ao[0:HD, :], pv0[0:HD, :])
                    ao1 = aopool.tile([128, IB], F16, tag="ao1", name="ao1")
                    nc.vector.tensor_copy(ao1[0:HD, :], pv1[0:HD, :])
                    nc.sync.dma_start(ao[64:64 + HD, :], ao1[0:HD, :])
                    aos.append(ao)

                # normalization: 1/d = exp(-ln d) on ACT + PE broadcast, all
                # deferred into a chunk so nothing blocks engine queues at the
                # i-block boundary.
                def norm_chunk(it=it, isl=isl, dd=dd, aos=aos):
                    lz = aopool.tile([128, 4 * IB], F32, tag="lz", name="lz",
                                     bufs=2)
                    nc.scalar.activation(lz[64:65, :], dd[64:65, :],
                                         mybir.ActivationFunctionType.Ln)
                    rr = aopool.tile([128, 4 * IB], MMDT, tag="rr", name="rr",
                                     bufs=2)
                    nc.scalar.activation(rr[64:65, :], lz[64:65, :],
                                         mybir.ActivationFunctionType.Exp,
                                         scale=-1.0)
                    for mt in range(2):
                        for h in range(2):
                            bc = ps_pj.tile([128, IB], F32, tag="pj", name="bc")
                            nc.tensor.matmul(
                                bc[:], onesr[64:65, 0:128],
                                rr[64:65, (2 * mt + h) * IB:(2 * mt + h + 1) * IB],
                                start=True, stop=True)
                            nc.vector.tensor_tensor(
                                aoT_sb[64 * h:64 * h + HD, mt, isl],
                                aos[mt][64 * h:64 * h + HD, :],
                                bc[64 * h:64 * h + HD, :], mybir.AluOpType.mult)
                return norm_chunk

            # ---- main pipeline (proj prefetch distance 2: proj(it) chunks
            # finish during it-2/it-1 so attention(it) never waits on rope) ----
            dummy = lambda: None  # noqa: E731
            pc0 = make_proj_chunks(0)   # xt block 0 DMA queued first
            load_consts_pre()
            pc1 = make_proj_chunks(1)   # prefetch xt(1) DMA now
            for c in pc0:
                c()
            load_consts_post()
            pc2 = make_proj_chunks(2)
            norm_prev = None
            for it in range(IBLK):
                chunks = []
                if it >= 3:
                    chunks += [dummy] * 3  # give it-1's norm DMAs time to land
                if norm_prev is not None:
                    chunks.append(norm_prev)
                if it == 0:
                    chunks += pc1 + pc2
                elif it + 2 < NBLK:
                    chunks += make_proj_chunks(it + 2)
                else:
                    chunks += [dummy] * 3
                if it >= 1:
                    chunks += make_oproj_chunks(it - 1)
                norm_prev = attention(it, chunks)
                for c in chunks:  # drain leftovers
                    c()
            norm_prev()
            for c in make_oproj_chunks(IBLK - 1, og_on_act=True):
                c()

    orig = nc.to_json_bytes
    nc.to_json_bytes = lambda: _split_multi_waits(orig())
    return nc


def _host_tables(position_ids):
    pos = np.asarray(position_ids).reshape(-1).astype(np.float64)
    inv = 1.0 / (ROPE_BASE ** (np.arange(0, HD, 2, dtype=np.float64) / HD))  # [32]
    fr = pos[None, :] * inv[:, None]                        # [32, S]
    c64 = np.concatenate([np.cos(fr), np.cos(fr)], axis=0)  # [64, S]
    s64 = np.concatenate([-np.sin(fr), np.sin(fr)], axis=0)  # rotate_half sign baked in
    cosT = np.vstack([c64, c64]).astype(MMNP)               # [128, S]
    sinT = np.vstack([s64, s64]).astype(MMNP)
    tri01 = np.where(np.arange(128)[:, None] <= np.arange(128)[None, :], 1.0, 0.0
                     ).astype(MMNP)
    return cosT, sinT, tri01


_NC_CACHE = {}


def kernel(**inputs):
    x = np.asarray(inputs["x"], dtype=np.float32)
    wq = np.asarray(inputs["wq"], dtype=np.float32)
    wk = np.asarray(inputs["wk"], dtype=np.float32)
    wv = np.asarray(inputs["wv"], dtype=np.float32)
    wo = np.asarray(inputs["wo"], dtype=np.float32)
    cosT, sinT, tri01 = _host_tables(inputs["position_ids"])
    xT = np.ascontiguousarray(x.reshape(S, H).T).astype(MMNP)

    if "nc" not in _NC_CACHE:
        _NC_CACHE["nc"] = build_nc()
    nc = _NC_CACHE["nc"]

    in_maps = []
    for c in range(NCORES):
        in_maps.append({
            "xT": xT,
            "wq": np.ascontiguousarray(wq[:, c * QW:(c + 1) * QW]).astype(MMNP),
            "wkv": np.ascontiguousarray(
                np.concatenate([wk[:, c * HD:(c + 1) * HD],
                                wv[:, c * HD:(c + 1) * HD]], axis=1)).astype(MMNP),
            "wo": np.ascontiguousarray(wo[c * QW:(c + 1) * QW, :]).astype(MMNP),
            "cosT": cosT, "sinT": sinT, "tri01": tri01,
            "ident64": np.eye(64, dtype=MMNP),
            "ones": np.ones((128, 129), dtype=MMNP),
        })
    res = run_bass_kernel_spmd(nc, in_maps, core_ids=list(range(NCORES)))
    acc = np.zeros((S, H), dtype=np.float32)
    for c in range(NCORES):
        acc += res.results[c]["out"].astype(np.float32)
    return acc.reshape(B, S, H)


if __name__ == "__main__":
    rng = np.random.default_rng(0)
    ins = {
        "x": rng.standard_normal((B, S, H), dtype=np.float32),
        "position_ids": np.broadcast_to(np.arange(S, dtype=np.int64), (B, S)),
        "wq": (rng.standard_normal((H, NH * HD), dtype=np.float32) * 0.02),
        "wk": (rng.standard_normal((H, NKV * HD), dtype=np.float32) * 0.02),
        "wv": (rng.standard_normal((H, NKV * HD), dtype=np.float32) * 0.02),
        "wo": (rng.standard_normal((NH * HD, H), dtype=np.float32) * 0.02),
    }
    out = kernel(**ins)
    print(out.shape, out.dtype, np.abs(out).mean())

